# revision 2
# baseline (speedup 1.0000x reference)
"""Trainium2 Bass kernel: 4-layer MLP (784-512-512-512-10) + log_softmax.

Data-parallel over 8 NeuronCores: batch 65536 is split into 8 shards of
8192 rows; the ~1M-param weights are replicated on every core.

v2 layout/schedule (over the 168us v1):
  * All DRAM buffers are host-packed so every DMA is contiguous per
    partition (KB-scale descriptors instead of 40B-1KB): x main
    [sc,128,6x1024], x remainder replicated into 4 row-groups, weights
    pre-swizzled, output written as the flat SBUF layout [128,64,10]
    and un-permuted on host.
  * Biases land first on the weight queue so the first ReLU never waits.
  * L1's K=16 remainder matmuls open each PSUM group via 4 row-tiled
    (tile_position=(32m,0)) concurrent matmuls instead of burning 8
    full 512-cycle slots at the end.
  * PSUM: one pool, 4 bufs x [128,2,512] = all 8 banks; layer-4 logits
    live in a 4th-buf slice.
  * Bias+ReLU is split per PSUM half: ScalarE takes hb0, DVE takes hb1
    (tensor_scalar add+max), halving the former 13.3us/superchunk
    ScalarE serial load that stalled the PE.
  * Layer 4 + log_softmax run one superchunk behind (software pipeline)
    so the PE never waits on L3 ReLUs; the per-superchunk epilogue
    (exp/sum/ln/sub + output DMA) replaces the old end-of-kernel burst
    that tailed 14us past the last matmul.
"""

from contextlib import ExitStack

import ml_dtypes
import numpy as np

import concourse.bass as bass  # noqa: F401  (registers AP machinery)
from concourse import bacc, mybir
from concourse.bass_utils import run_bass_kernel_spmd
from concourse.tile import TileContext

BF16 = mybir.dt.bfloat16
FP32 = mybir.dt.float32
FP8 = mybir.dt.float8e4

N_CORES = 8
B = 65536
D0, H, C = 784, 512, 10
BC = B // N_CORES            # 8192 rows per core
NB = 512                     # matmul moving free dim / PSUM bank width
HB = 2                       # batch halves sharing one PSUM group
SNB = NB * HB                # 1024-row superchunk
NCHUNK = BC // SNB           # 8 superchunks
K0F = 6                      # full 128-row contraction chunks in layer 1
K0R = D0 - K0F * 128         # 16 remainder rows
KH = H // 128                # 4 contraction chunks for hidden layers
MG = SNB // 128              # 8 row-groups per superchunk
NRG = BC // 128              # 64 row-groups of 128 rows per core

_CACHED_NC = None


def build_nc():
    nc = bacc.Bacc(
        "TRN2",
        target_bir_lowering=False,
        debug=False,
        enable_asserts=False,
        num_devices=N_CORES,
    )
    xm_d = nc.declare_dram_parameter("xmain", [NCHUNK * 128, K0F * SNB], FP8, isOutput=False)
    xr_d = nc.declare_dram_parameter("xrem", [NCHUNK * 128, SNB], FP8, isOutput=False)
    w1_d = nc.declare_dram_parameter("w1p", [128, K0F * H], FP8, isOutput=False)
    w1r_d = nc.declare_dram_parameter("w1r", [128, 128], FP8, isOutput=False)
    w2_d = nc.declare_dram_parameter("w2p", [128, KH * H], FP8, isOutput=False)
    w3_d = nc.declare_dram_parameter("w3p", [128, KH * H], FP8, isOutput=False)
    w4_d = nc.declare_dram_parameter("w4p", [128, KH * C], BF16, isOutput=False)
    bal_d = nc.declare_dram_parameter("ball", [128, 3 * KH + C], FP32, isOutput=False)
    out_d = nc.declare_dram_parameter("out", [128, NRG, C], FP32, isOutput=True)

    expf = mybir.ActivationFunctionType.Exp
    reluf = mybir.ActivationFunctionType.Relu
    lnf = mybir.ActivationFunctionType.Ln
    add_op = mybir.AluOpType.add
    max_op = mybir.AluOpType.max
    sub_op = mybir.AluOpType.subtract
    drow = mybir.MatmulPerfMode.DoubleRow

    with TileContext(nc) as tc, ExitStack() as ctx:
        consts = ctx.enter_context(tc.tile_pool(name="consts", bufs=1))
        xpool = ctx.enter_context(tc.tile_pool(name="xp", bufs=3))
        hpool = ctx.enter_context(tc.tile_pool(name="hp", bufs=3))
        spool = ctx.enter_context(tc.tile_pool(name="sp", bufs=2))
        pbig = ctx.enter_context(tc.tile_pool(name="pbig", bufs=4, space="PSUM"))

        # Biases first: the first ReLU depends on them, and they are tiny.
        ball = consts.tile([128, 3 * KH + C], FP32, tag="ball", name="ball")
        nc.scalar.dma_start(ball[:], bal_d[:])
        # w1 remainder + row-tiled remainder weights next (first matmuls).
        w1r = consts.tile([128, 128], FP8, tag="w1r", name="w1r")
        nc.scalar.dma_start(w1r[:], w1r_d[:])
        w1 = consts.tile([128, K0F, H], FP8, tag="w1", name="w1")
        nc.scalar.dma_start(w1[:], w1_d[:])
        w2 = consts.tile([128, KH, H], FP8, tag="w2", name="w2")
        nc.scalar.dma_start(w2[:], w2_d[:])
        w3 = consts.tile([128, KH, H], FP8, tag="w3", name="w3")
        nc.scalar.dma_start(w3[:], w3_d[:])
        w4 = consts.tile([128, KH, C], BF16, tag="w4", name="w4")
        nc.scalar.dma_start(w4[:], w4_d[:])

        b4s = ball[:, 3 * KH : 3 * KH + C]

        # PE warm-up: dummy matmuls during the initial DMA wait so the HAM
        # clock gate is at 2.4 GHz when real work arrives.
        warm = consts.tile([128, NB], FP8, tag="warm", name="warm")
        nc.vector.memset(warm[:], 1.0)
        psw = pbig.tile([128, HB, NB], FP32, tag="ps", name="ps_warm")
        for i in range(14):
            nc.tensor.matmul(
                psw[:, i % 2, :], lhsT=warm[:, 0:128], rhs=warm[:],
                start=(i < 2), stop=(i >= 12),
            )

        def relu_pair(ps, out0, out1, bias_ap):
            # bias+ReLU of one [128,2,512] PSUM group: ScalarE takes half 0,
            # DVE takes half 1, so both halves finish ~0.7us after the stop.
            nc.scalar.activation(out0, ps[:, 0, :], reluf, bias=bias_ap)
            nc.vector.tensor_scalar(out1, ps[:, 1, :], bias_ap, 0.0, add_op, max_op)

        h3_prev = None
        ps4_prev = None

        def l4_and_epilogue(sc, h3, ps4):
            # Layer 4 [512 -> 10] (bf16, batch on PSUM partitions) and the
            # full log_softmax epilogue for superchunk sc.
            for hb in range(HB):
                for mm in range(NB // 128):
                    r = hb * (NB // 128) + mm
                    ms = slice(mm * 128, (mm + 1) * 128)
                    for k in range(KH):
                        nc.tensor.matmul(
                            ps4[:, r, :], lhsT=h3[k][:, hb, ms], rhs=w4[:, k, :],
                            start=(k == 0), stop=(k == KH - 1),
                        )
            lg = spool.tile([128, MG, C], FP32, tag="lg", name="lg")
            nc.vector.tensor_tensor(
                lg[:], ps4[:, 0:MG, :],
                b4s[:, None, :].to_broadcast((128, MG, C)), add_op,
            )
            etile = spool.tile([128, MG, C], FP32, tag="etile", name="etile")
            nc.scalar.activation(etile[:], lg[:], expf)
            esum = spool.tile([128, MG], FP32, tag="esum", name="esum")
            nc.vector.tensor_reduce(
                esum[:], etile[:], axis=mybir.AxisListType.X, op=add_op
            )
            lns = spool.tile([128, MG], FP32, tag="lns", name="lns")
            nc.scalar.activation(lns[:], esum[:], lnf)
            obuf = spool.tile([128, MG, C], FP32, tag="obuf", name="obuf")
            nc.vector.tensor_tensor(
                obuf[:], lg[:],
                lns[:, :, None].to_broadcast((128, MG, C)), sub_op,
            )
            nc.sync.dma_start(out_d[:, sc * MG : (sc + 1) * MG, :], obuf[:])

        for sc in range(NCHUNK):
            xr = xpool.tile([128, SNB], FP8, tag="xr", name="xr")
            nc.sync.dma_start(xr[:], xr_d[sc * 128 : (sc + 1) * 128, :])
            xt = xpool.tile([128, K0F, SNB], FP8, tag="xt", name="xt")
            nc.sync.dma_start(xt[:], xm_d[sc * 128 : (sc + 1) * 128, :])

            # Layer 1 [784 -> 512]: open each m-chunk's PSUM group with the
            # K=16 remainder in row-group m (4 groups run concurrently),
            # then 3 fp8-DoubleRow slots per (m, hb).
            ps1 = [
                pbig.tile([128, HB, NB], FP32, tag="ps", name=f"ps1_{m}")
                for m in range(KH)
            ]
            for hb in range(HB):
                for m in range(KH):
                    nc.tensor.matmul(
                        ps1[m][:, hb, :],
                        lhsT=w1r[32 * m : 32 * m + K0R, :],
                        rhs=xr[32 * m : 32 * m + K0R, hb * NB : (hb + 1) * NB],
                        start=True, stop=False, perf_mode=None,
                        tile_position=(32 * m, 0),
                    )
            h1p = [
                hpool.tile([128, 2, HB, NB], FP8, tag=f"h1p_{j}", name=f"h1p_{j}")
                for j in range(KH // 2)
            ]
            for m in range(KH):
                ms = slice(m * 128, (m + 1) * 128)
                for k in range(0, K0F, 2):
                    for hb in range(HB):
                        nc.tensor.matmul(
                            ps1[m][:, hb, :], lhsT=w1[:, k : k + 2, ms],
                            rhs=xt[:, k : k + 2, hb * NB : (hb + 1) * NB],
                            start=False, stop=(k == K0F - 2), perf_mode=drow,
                        )
                relu_pair(
                    ps1[m],
                    h1p[m // 2][:, m % 2, 0, :], h1p[m // 2][:, m % 2, 1, :],
                    ball[:, m : m + 1],
                )

            # Layer 4 + epilogue of the previous superchunk: its h3 ReLUs
            # finished long ago, so these matmuls never stall; the epilogue
            # engine work hides under layer 2.
            if h3_prev is not None:
                l4_and_epilogue(sc - 1, h3_prev, ps4_prev)

            # Layer 2 [512 -> 512]: fp8 DoubleRow over feature-chunk pairs.
            h2p = [
                hpool.tile([128, 2, HB, NB], FP8, tag=f"h2p_{j}", name=f"h2p_{j}")
                for j in range(KH // 2)
            ]
            ps2 = [
                pbig.tile([128, HB, NB], FP32, tag="ps", name=f"ps2_{m}")
                for m in range(KH)
            ]
            for m in range(KH):
                ms = slice(m * 128, (m + 1) * 128)
                for j in range(KH // 2):
                    for hb in range(HB):
                        nc.tensor.matmul(
                            ps2[m][:, hb, :], lhsT=w2[:, 2 * j : 2 * j + 2, ms],
                            rhs=h1p[j][:, :, hb, :],
                            start=(j == 0), stop=(j == KH // 2 - 1),
                            perf_mode=drow,
                        )
                relu_pair(
                    ps2[m],
                    h2p[m // 2][:, m % 2, 0, :], h2p[m // 2][:, m % 2, 1, :],
                    ball[:, KH + m : KH + m + 1],
                )

            # Layer 3 [512 -> 512]: fp8 DoubleRow in, bf16 out (layer-4 lhsT).
            h3 = [
                hpool.tile([128, HB, NB], BF16, tag=f"h3_{m}", name=f"h3_{m}")
                for m in range(KH)
            ]
            ps3 = [
                pbig.tile([128, HB, NB], FP32, tag="ps", name=f"ps3_{m}")
                for m in range(KH)
            ]
            for m in range(KH):
                ms = slice(m * 128, (m + 1) * 128)
                for j in range(KH // 2):
                    for hb in range(HB):
                        nc.tensor.matmul(
                            ps3[m][:, hb, :], lhsT=w3[:, 2 * j : 2 * j + 2, ms],
                            rhs=h2p[j][:, :, hb, :],
                            start=(j == 0), stop=(j == KH // 2 - 1),
                            perf_mode=drow,
                        )
                relu_pair(
                    ps3[m],
                    h3[m][:, 0, :], h3[m][:, 1, :],
                    ball[:, 2 * KH + m : 2 * KH + m + 1],
                )

            h3_prev = h3
            ps4_prev = pbig.tile([128, 4 * MG, C], FP32, tag="ps", name="ps4")

        l4_and_epilogue(NCHUNK - 1, h3_prev, ps4_prev)

    nc.compile()
    return nc


def _get_nc():
    global _CACHED_NC
    if _CACHED_NC is None:
        _CACHED_NC = build_nc()
    return _CACHED_NC


def make_in_maps(x, W1, b1, W2, b2, W3, b3, W4, b4):
    bf16 = ml_dtypes.bfloat16
    fp8 = ml_dtypes.float8_e4m3
    f32 = np.float32
    W1, W2, W3, W4 = (np.asarray(w, dtype=f32) for w in (W1, W2, W3, W4))

    # w1p[p, k*512+m] = W1[m, k*128+p]
    w1p = np.ascontiguousarray(
        W1[:, : K0F * 128].reshape(H, K0F, 128).transpose(2, 1, 0)
    ).reshape(128, K0F * H).astype(fp8)
    # w1r[32i+j, c] = W1[128i+c, 768+j]  (row-tiled remainder weights)
    w1r = np.zeros((128, 128), dtype=fp8)
    wr = W1[:, K0F * 128 :].astype(fp8)  # [512, 16]
    for i in range(KH):
        w1r[32 * i : 32 * i + K0R, :] = wr[128 * i : 128 * (i + 1), :].T
    # w2p[p, o*512+m] = W2[m, o*128+p]
    def packw(W):
        return np.ascontiguousarray(
            W.T.reshape(KH, 128, H).transpose(1, 0, 2)
        ).reshape(128, KH * H).astype(fp8)
    w2p, w3p = packw(W2), packw(W3)
    w4p = np.ascontiguousarray(
        W4.T.reshape(KH, 128, C).transpose(1, 0, 2)
    ).reshape(128, KH * C).astype(bf16)
    ball = np.concatenate(
        [
            np.asarray(b1, f32).reshape(KH, 128).T,
            np.asarray(b2, f32).reshape(KH, 128).T,
            np.asarray(b3, f32).reshape(KH, 128).T,
            np.tile(np.asarray(b4, f32)[None, :], (128, 1)),
        ],
        axis=1,
    )
    common = {
        "w1p": w1p, "w1r": w1r, "w2p": w2p, "w3p": w3p, "w4p": w4p,
        "ball": np.ascontiguousarray(ball),
    }

    xq = np.asarray(x).astype(fp8)
    in_maps = []
    for ci in range(N_CORES):
        xs = xq[ci * BC : (ci + 1) * BC]  # [8192, 784]
        # xmain[sc*128+p, k*1024+b] = xs[sc*1024+b, k*128+p]
        xmain = np.ascontiguousarray(
            xs[:, : K0F * 128].reshape(NCHUNK, SNB, K0F, 128).transpose(0, 3, 2, 1)
        ).reshape(NCHUNK * 128, K0F * SNB)
        # xrem[sc*128+32i+j, b] = xs[sc*1024+b, 768+j], replicated over i
        xrp = xs[:, K0F * 128 :].reshape(NCHUNK, SNB, K0R).transpose(0, 2, 1)
        xrem = np.zeros((NCHUNK, 128, SNB), dtype=fp8)
        for i in range(KH):
            xrem[:, 32 * i : 32 * i + K0R, :] = xrp
        in_maps.append(
            {"xmain": xmain, "xrem": xrem.reshape(NCHUNK * 128, SNB), **common}
        )
    return in_maps


def assemble_output(res):
    # out dram is the flat SBUF layout [128, 64, 10]; row rg*128+p of the
    # core's shard lives at out[p, rg, :].
    parts = []
    for i in range(N_CORES):
        o = np.asarray(res.results[i]["out"], dtype=np.float32)
        parts.append(o.transpose(1, 0, 2).reshape(BC, C))
    return np.concatenate(parts, axis=0)


def kernel(x, W1, b1, W2, b2, W3, b3, W4, b4):
    in_maps = make_in_maps(x, W1, b1, W2, b2, W3, b3, W4, b4)
    nc = _get_nc()
    res = run_bass_kernel_spmd(nc, in_maps, list(range(N_CORES)))
    return assemble_output(res)


# revision 6
# speedup vs baseline: 1.1438x; 1.1438x over previous
"""Trainium2 Bass kernel: 4-layer MLP (784-512-512-512-10) + log_softmax.

Data-parallel over 8 NeuronCores: batch 65536 is split into 8 shards of
8192 rows; the ~1M-param weights are replicated on every core.

v2 layout/schedule (over the 168us v1):
  * All DRAM buffers are host-packed so every DMA is contiguous per
    partition (KB-scale descriptors instead of 40B-1KB): x main
    [sc,128,6x1024], x remainder replicated into 4 row-groups, weights
    pre-swizzled, output written as the flat SBUF layout [128,64,10]
    and un-permuted on host.
  * Biases land first on the weight queue so the first ReLU never waits.
  * L1's K=16 remainder matmuls open each PSUM group via 4 row-tiled
    (tile_position=(32m,0)) concurrent matmuls instead of burning 8
    full 512-cycle slots at the end.
  * PSUM: one pool, 4 bufs x [128,2,512] = all 8 banks; layer-4 logits
    live in a 4th-buf slice.
  * Bias+ReLU is split per PSUM half: ScalarE takes hb0, DVE takes hb1
    (tensor_scalar add+max), halving the former 13.3us/superchunk
    ScalarE serial load that stalled the PE.
  * Layer 4 + log_softmax run one superchunk behind (software pipeline)
    so the PE never waits on L3 ReLUs; the per-superchunk epilogue
    (exp/sum/ln/sub + output DMA) replaces the old end-of-kernel burst
    that tailed 14us past the last matmul.
"""

from contextlib import ExitStack

import ml_dtypes
import numpy as np

import concourse.bass as bass  # noqa: F401  (registers AP machinery)
from concourse import bacc, mybir
from concourse.bass_utils import run_bass_kernel_spmd
from concourse.tile import TileContext

BF16 = mybir.dt.bfloat16
FP32 = mybir.dt.float32
FP8 = mybir.dt.float8e4

N_CORES = 8
B = 65536
D0, H, C = 784, 512, 10
BC = B // N_CORES            # 8192 rows per core
NB = 512                     # matmul moving free dim / PSUM bank width
HB = 2                       # batch halves sharing one PSUM group
SNB = NB * HB                # 1024-row superchunk
NCHUNK = BC // SNB           # 8 superchunks
K0F = 6                      # full 128-row contraction chunks in layer 1
K0R = D0 - K0F * 128         # 16 remainder rows
KH = H // 128                # 4 contraction chunks for hidden layers
MG = SNB // 128              # 8 row-groups per superchunk
NRG = BC // 128              # 64 row-groups of 128 rows per core

_CACHED_NC = None


def build_nc():
    nc = bacc.Bacc(
        "TRN2",
        target_bir_lowering=False,
        debug=False,
        enable_asserts=False,
        num_devices=N_CORES,
    )
    xm_d = nc.declare_dram_parameter("xmain", [NCHUNK * 128, K0F * SNB], FP8, isOutput=False)
    xr_d = nc.declare_dram_parameter("xrem", [NCHUNK * 128, SNB], FP8, isOutput=False)
    w1_d = nc.declare_dram_parameter("w1p", [128, K0F * H], FP8, isOutput=False)
    w1r_d = nc.declare_dram_parameter("w1r", [128, 128], FP8, isOutput=False)
    w2_d = nc.declare_dram_parameter("w2p", [128, KH * H], FP8, isOutput=False)
    w3_d = nc.declare_dram_parameter("w3p", [128, KH * H], FP8, isOutput=False)
    w4_d = nc.declare_dram_parameter("w4p", [128, KH * C], BF16, isOutput=False)
    bal_d = nc.declare_dram_parameter("ball", [128, 3 * KH + C], FP32, isOutput=False)
    out_d = nc.declare_dram_parameter("out", [128, NRG, C], FP32, isOutput=True)

    expf = mybir.ActivationFunctionType.Exp
    reluf = mybir.ActivationFunctionType.Relu
    lnf = mybir.ActivationFunctionType.Ln
    add_op = mybir.AluOpType.add
    max_op = mybir.AluOpType.max
    sub_op = mybir.AluOpType.subtract
    drow = mybir.MatmulPerfMode.DoubleRow

    with TileContext(nc) as tc, ExitStack() as ctx:
        consts = ctx.enter_context(tc.tile_pool(name="consts", bufs=1))
        xpool = ctx.enter_context(tc.tile_pool(name="xp", bufs=3))
        hpool = ctx.enter_context(tc.tile_pool(name="hp", bufs=3))
        spool = ctx.enter_context(tc.tile_pool(name="sp", bufs=2))
        pbig = ctx.enter_context(tc.tile_pool(name="pbig", bufs=4, space="PSUM"))

        # Weight-queue order = first-use order: the row-tiled remainder
        # weights open superchunk 0, then w1, then biases (first ReLU is
        # ~2us later), then the rest.
        w1r = consts.tile([128, 128], FP8, tag="w1r", name="w1r")
        nc.scalar.dma_start(w1r[:], w1r_d[:])
        w1 = consts.tile([128, K0F, H], FP8, tag="w1", name="w1")
        nc.scalar.dma_start(w1[:], w1_d[:])
        ball = consts.tile([128, 3 * KH + C], FP32, tag="ball", name="ball")
        nc.scalar.dma_start(ball[:], bal_d[:])
        w2 = consts.tile([128, KH, H], FP8, tag="w2", name="w2")
        nc.scalar.dma_start(w2[:], w2_d[:])
        w3 = consts.tile([128, KH, H], FP8, tag="w3", name="w3")
        nc.scalar.dma_start(w3[:], w3_d[:])
        w4 = consts.tile([128, KH, C], BF16, tag="w4", name="w4")
        nc.scalar.dma_start(w4[:], w4_d[:])

        b4s = ball[:, 3 * KH : 3 * KH + C]

        # PE warm-up: dummy matmuls during the initial DMA wait so the HAM
        # clock gate is at 2.4 GHz when real work arrives.
        warm = consts.tile([128, NB], FP8, tag="warm", name="warm")
        nc.vector.memset(warm[:], 1.0)
        psw = pbig.tile([128, HB, NB], FP32, tag="ps", name="ps_warm")
        for i in range(6):
            nc.tensor.matmul(
                psw[:, i % 2, :], lhsT=warm[:, 0:128], rhs=warm[:],
                start=(i < 2), stop=(i >= 4),
            )

        # Persistent softmax state: ln+subtract are deferred to two bulk
        # epilogue calls, so ScalarE never swaps activation tables (RELU/EXP
        # share a set, LN does not) inside the steady-state loop.
        logits_all = consts.tile([128, NRG, C], FP32, tag="logits_all", name="logits_all")
        esum_all = consts.tile([128, NRG], FP32, tag="esum_all", name="esum_all")
        lns_all = consts.tile([128, NRG], FP32, tag="lns_all", name="lns_all")
        obuf = consts.tile([128, NRG, C], FP32, tag="obuf", name="obuf")

        def relu_pair(ps, out0, out1, bias_ap):
            # bias+ReLU of one [128,2,512] PSUM group: ScalarE takes half 0,
            # DVE takes half 1, so both halves finish ~0.7us after the stop.
            nc.scalar.activation(out0, ps[:, 0, :], reluf, bias=bias_ap)
            nc.vector.tensor_scalar(out1, ps[:, 1, :], bias_ap, 0.0, add_op, max_op)

        h3_prev = None
        ps4_prev = None

        def l4_matmuls(h3, ps4):
            for hb in range(HB):
                for mm in range(NB // 128):
                    r = hb * (NB // 128) + mm
                    ms = slice(mm * 128, (mm + 1) * 128)
                    for k in range(KH):
                        nc.tensor.matmul(
                            ps4[:, r, :], lhsT=h3[k][:, hb, ms], rhs=w4[:, k, :],
                            start=(k == 0), stop=(k == KH - 1),
                        )

        def l4_softmax_state(sc, ps4):
            # logits + exp + sum(exp) for superchunk sc (issued at the end
            # of the next superchunk's block so it never delays ReLUs).
            rg0 = sc * MG
            lg = logits_all[:, rg0 : rg0 + MG, :]
            nc.vector.tensor_tensor(
                lg, ps4[:, 0:MG, :],
                b4s[:, None, :].to_broadcast((128, MG, C)), add_op,
            )
            etile = spool.tile([128, MG, C], FP32, tag="etile", name="etile")
            nc.scalar.activation(etile[:], lg, expf)
            nc.vector.tensor_reduce(
                esum_all[:, rg0 : rg0 + MG], etile[:],
                axis=mybir.AxisListType.X, op=add_op,
            )

        def softmax_epilogue(rg0, rg1):
            # out = logits - ln(sum(exp(logits))) for row-groups [rg0, rg1)
            n = rg1 - rg0
            nc.scalar.activation(lns_all[:, rg0:rg1], esum_all[:, rg0:rg1], lnf)
            nc.vector.tensor_tensor(
                obuf[:, rg0:rg1, :], logits_all[:, rg0:rg1, :],
                lns_all[:, rg0:rg1, None].to_broadcast((128, n, C)), sub_op,
            )
            nc.sync.dma_start(out_d[:, rg0:rg1, :], obuf[:, rg0:rg1, :])

        for sc in range(NCHUNK):
            xr = xpool.tile([128, SNB], FP8, tag="xr", name="xr")
            nc.sync.dma_start(xr[:], xr_d[sc * 128 : (sc + 1) * 128, :])
            xt = xpool.tile([128, K0F, SNB], FP8, tag="xt", name="xt")
            nc.sync.dma_start(xt[:], xm_d[sc * 128 : (sc + 1) * 128, :])

            # Layer 1 [784 -> 512]: open each m-chunk's PSUM group with the
            # K=16 remainder in row-group m (4 groups run concurrently),
            # then 3 fp8-DoubleRow slots per (m, hb).
            ps1 = [
                pbig.tile([128, HB, NB], FP32, tag="ps", name=f"ps1_{m}")
                for m in range(KH)
            ]
            for hb in range(HB):
                for m in range(KH):
                    nc.tensor.matmul(
                        ps1[m][:, hb, :],
                        lhsT=w1r[32 * m : 32 * m + K0R, :],
                        rhs=xr[32 * m : 32 * m + K0R, hb * NB : (hb + 1) * NB],
                        start=True, stop=False, perf_mode=None,
                        tile_position=(32 * m, 0),
                    )
            h1p = [
                hpool.tile([128, 2, HB, NB], FP8, tag=f"h1p_{j}", name=f"h1p_{j}")
                for j in range(KH // 2)
            ]
            for m in range(KH):
                ms = slice(m * 128, (m + 1) * 128)
                for k in range(0, K0F, 2):
                    for hb in range(HB):
                        nc.tensor.matmul(
                            ps1[m][:, hb, :], lhsT=w1[:, k : k + 2, ms],
                            rhs=xt[:, k : k + 2, hb * NB : (hb + 1) * NB],
                            start=False, stop=(k == K0F - 2), perf_mode=drow,
                        )
                relu_pair(
                    ps1[m],
                    h1p[m // 2][:, m % 2, 0, :], h1p[m // 2][:, m % 2, 1, :],
                    ball[:, m : m + 1],
                )

            # Layer 4 of the previous superchunk: its h3 ReLUs finished long
            # ago, so these matmuls never stall, and they give layer 1's
            # last ReLU time to land before layer 2 consumes it.
            if h3_prev is not None:
                l4_matmuls(h3_prev, ps4_prev)

            # Layer 2 [512 -> 512]: fp8 DoubleRow over feature-chunk pairs.
            h2p = [
                hpool.tile([128, 2, HB, NB], FP8, tag=f"h2p_{j}", name=f"h2p_{j}")
                for j in range(KH // 2)
            ]
            ps2 = [
                pbig.tile([128, HB, NB], FP32, tag="ps", name=f"ps2_{m}")
                for m in range(KH)
            ]
            for m in range(KH):
                ms = slice(m * 128, (m + 1) * 128)
                for j in range(KH // 2):
                    for hb in range(HB):
                        nc.tensor.matmul(
                            ps2[m][:, hb, :], lhsT=w2[:, 2 * j : 2 * j + 2, ms],
                            rhs=h1p[j][:, :, hb, :],
                            start=(j == 0), stop=(j == KH // 2 - 1),
                            perf_mode=drow,
                        )
                relu_pair(
                    ps2[m],
                    h2p[m // 2][:, m % 2, 0, :], h2p[m // 2][:, m % 2, 1, :],
                    ball[:, KH + m : KH + m + 1],
                )

            # Layer 3 [512 -> 512]: fp8 DoubleRow in, bf16 out (layer-4 lhsT).
            h3 = [
                hpool.tile([128, HB, NB], BF16, tag=f"h3_{m}", name=f"h3_{m}")
                for m in range(KH)
            ]
            ps3 = [
                pbig.tile([128, HB, NB], FP32, tag="ps", name=f"ps3_{m}")
                for m in range(KH)
            ]
            for m in range(KH):
                ms = slice(m * 128, (m + 1) * 128)
                for j in range(KH // 2):
                    for hb in range(HB):
                        nc.tensor.matmul(
                            ps3[m][:, hb, :], lhsT=w3[:, 2 * j : 2 * j + 2, ms],
                            rhs=h2p[j][:, :, hb, :],
                            start=(j == 0), stop=(j == KH // 2 - 1),
                            perf_mode=drow,
                        )
                relu_pair(
                    ps3[m],
                    h3[m][:, 0, :], h3[m][:, 1, :],
                    ball[:, 2 * KH + m : 2 * KH + m + 1],
                )

            # exp/sum(exp) of the previous superchunk, issued last so the
            # ScalarE/DVE queues drain all of this superchunk's ReLUs first.
            if sc > 0:
                l4_softmax_state(sc - 1, ps4_prev)
            if sc == NCHUNK - 2:
                # Bulk ln+subtract+store for superchunks 0-5 hides under the
                # last superchunk's matmuls.
                softmax_epilogue(0, (NCHUNK - 2) * MG)

            h3_prev = h3
            ps4_prev = pbig.tile([128, 4 * MG, C], FP32, tag="ps", name="ps4")

        l4_matmuls(h3_prev, ps4_prev)
        l4_softmax_state(NCHUNK - 1, ps4_prev)
        softmax_epilogue((NCHUNK - 2) * MG, NRG)

    nc.compile()
    return nc


def _get_nc():
    global _CACHED_NC
    if _CACHED_NC is None:
        _CACHED_NC = build_nc()
    return _CACHED_NC


def make_in_maps(x, W1, b1, W2, b2, W3, b3, W4, b4):
    bf16 = ml_dtypes.bfloat16
    fp8 = ml_dtypes.float8_e4m3
    f32 = np.float32
    W1, W2, W3, W4 = (np.asarray(w, dtype=f32) for w in (W1, W2, W3, W4))

    # w1p[p, k*512+m] = W1[m, k*128+p]
    w1p = np.ascontiguousarray(
        W1[:, : K0F * 128].reshape(H, K0F, 128).transpose(2, 1, 0)
    ).reshape(128, K0F * H).astype(fp8)
    # w1r[32i+j, c] = W1[128i+c, 768+j]  (row-tiled remainder weights)
    w1r = np.zeros((128, 128), dtype=fp8)
    wr = W1[:, K0F * 128 :].astype(fp8)  # [512, 16]
    for i in range(KH):
        w1r[32 * i : 32 * i + K0R, :] = wr[128 * i : 128 * (i + 1), :].T
    # w2p[p, o*512+m] = W2[m, o*128+p]
    def packw(W):
        return np.ascontiguousarray(
            W.T.reshape(KH, 128, H).transpose(1, 0, 2)
        ).reshape(128, KH * H).astype(fp8)
    w2p, w3p = packw(W2), packw(W3)
    w4p = np.ascontiguousarray(
        W4.T.reshape(KH, 128, C).transpose(1, 0, 2)
    ).reshape(128, KH * C).astype(bf16)
    ball = np.concatenate(
        [
            np.asarray(b1, f32).reshape(KH, 128).T,
            np.asarray(b2, f32).reshape(KH, 128).T,
            np.asarray(b3, f32).reshape(KH, 128).T,
            np.tile(np.asarray(b4, f32)[None, :], (128, 1)),
        ],
        axis=1,
    )
    common = {
        "w1p": w1p, "w1r": w1r, "w2p": w2p, "w3p": w3p, "w4p": w4p,
        "ball": np.ascontiguousarray(ball),
    }

    xq = np.asarray(x).astype(fp8)
    in_maps = []
    for ci in range(N_CORES):
        xs = xq[ci * BC : (ci + 1) * BC]  # [8192, 784]
        # xmain[sc*128+p, k*1024+b] = xs[sc*1024+b, k*128+p]
        xmain = np.ascontiguousarray(
            xs[:, : K0F * 128].reshape(NCHUNK, SNB, K0F, 128).transpose(0, 3, 2, 1)
        ).reshape(NCHUNK * 128, K0F * SNB)
        # xrem[sc*128+32i+j, b] = xs[sc*1024+b, 768+j], replicated over i
        xrp = xs[:, K0F * 128 :].reshape(NCHUNK, SNB, K0R).transpose(0, 2, 1)
        xrem = np.zeros((NCHUNK, 128, SNB), dtype=fp8)
        for i in range(KH):
            xrem[:, 32 * i : 32 * i + K0R, :] = xrp
        in_maps.append(
            {"xmain": xmain, "xrem": xrem.reshape(NCHUNK * 128, SNB), **common}
        )
    return in_maps


def assemble_output(res):
    # out dram is the flat SBUF layout [128, 64, 10]; row rg*128+p of the
    # core's shard lives at out[p, rg, :].
    parts = []
    for i in range(N_CORES):
        o = np.asarray(res.results[i]["out"], dtype=np.float32)
        parts.append(o.transpose(1, 0, 2).reshape(BC, C))
    return np.concatenate(parts, axis=0)


def kernel(x, W1, b1, W2, b2, W3, b3, W4, b4):
    in_maps = make_in_maps(x, W1, b1, W2, b2, W3, b3, W4, b4)
    nc = _get_nc()
    res = run_bass_kernel_spmd(nc, in_maps, list(range(N_CORES)))
    return assemble_output(res)


# revision 7
# speedup vs baseline: 1.2777x; 1.1170x over previous
"""Trainium2 Bass kernel: 4-layer MLP (784-512-512-512-10) + log_softmax.

Data-parallel over 8 NeuronCores: batch 65536 is split into 8 shards of
8192 rows; the ~1M-param weights are replicated on every core.

v4 schedule: batch-half-major pipeline with 1-bank PSUM groups.
  * Every (m-chunk, batch-half) matmul group accumulates into its own
    2KB PSUM bank (pool of 8); consumers run one half-block (~3us)
    behind producers, so neither the PE nor its LDWEIGHTS ever waits on
    a ReLU drain (the v3 failure mode: 4 simultaneously-opened 2-bank
    groups stalled the in-order PE queue on PSUM WAR ~1.5us/superchunk
    and the micro-gaps re-throttled the clock to 1.2 GHz).
  * Layer 1's K=16 remainder opens each group via row-tiled
    (tile_position=(32m,0)) matmuls, 4 running concurrently in distinct
    PE row-groups, instead of burning 8 full 512-cycle slots.
  * bias+ReLU alternates ScalarE/DVE per (m, half) so both engines stay
    under the PE's pace; exp/sum-of-exp for superchunk sc-1 is issued
    mid-superchunk (never ahead of ReLUs the PE needs); ln+subtract+
    store happen in two bulk epilogues (ScalarE activation-table swaps
    for LN cost 1.3us each, so they must not recur per superchunk).
  * Layer 4 runs one superchunk behind; all DRAM buffers host-packed
    for contiguous-per-partition DMA; output leaves in the flat SBUF
    layout [128, 64, 10] and is un-permuted on host.

Measured on axon trn2: 172.8us (v1 baseline) -> 155.2us (v3) -> v4.
"""

from contextlib import ExitStack

import ml_dtypes
import numpy as np

import concourse.bass as bass  # noqa: F401  (registers AP machinery)
from concourse import bacc, mybir
from concourse.bass_utils import run_bass_kernel_spmd
from concourse.tile import TileContext

BF16 = mybir.dt.bfloat16
FP32 = mybir.dt.float32
FP8 = mybir.dt.float8e4

N_CORES = 8
B = 65536
D0, H, C = 784, 512, 10
BC = B // N_CORES            # 8192 rows per core
NB = 512                     # matmul moving free dim / PSUM bank width
HB = 2                       # batch halves per superchunk
SNB = NB * HB                # 1024-row superchunk
NCHUNK = BC // SNB           # 8 superchunks
K0F = 6                      # full 128-row contraction chunks in layer 1
K0R = D0 - K0F * 128         # 16 remainder rows
KH = H // 128                # 4 contraction chunks for hidden layers
MG = SNB // 128              # 8 row-groups per superchunk
NRG = BC // 128              # 64 row-groups of 128 rows per core

_CACHED_NC = None


def build_nc():
    nc = bacc.Bacc(
        "TRN2",
        target_bir_lowering=False,
        debug=False,
        enable_asserts=False,
        num_devices=N_CORES,
    )
    xm_d = nc.declare_dram_parameter("xmain", [NCHUNK * 128, K0F * SNB], FP8, isOutput=False)
    xr_d = nc.declare_dram_parameter("xrem", [NCHUNK * 128, SNB], FP8, isOutput=False)
    w1_d = nc.declare_dram_parameter("w1p", [128, K0F * H], FP8, isOutput=False)
    w1r_d = nc.declare_dram_parameter("w1r", [128, 128], FP8, isOutput=False)
    w2_d = nc.declare_dram_parameter("w2p", [128, KH * H], FP8, isOutput=False)
    w3_d = nc.declare_dram_parameter("w3p", [128, KH * H], FP8, isOutput=False)
    w4_d = nc.declare_dram_parameter("w4p", [128, KH * C], BF16, isOutput=False)
    bal_d = nc.declare_dram_parameter("ball", [128, 3 * KH + C], FP32, isOutput=False)
    out_d = nc.declare_dram_parameter("out", [128, NRG, C], FP32, isOutput=True)

    expf = mybir.ActivationFunctionType.Exp
    reluf = mybir.ActivationFunctionType.Relu
    lnf = mybir.ActivationFunctionType.Ln
    add_op = mybir.AluOpType.add
    max_op = mybir.AluOpType.max
    sub_op = mybir.AluOpType.subtract
    drow = mybir.MatmulPerfMode.DoubleRow

    with TileContext(nc) as tc, ExitStack() as ctx:
        consts = ctx.enter_context(tc.tile_pool(name="consts", bufs=1))
        xpool = ctx.enter_context(tc.tile_pool(name="xp", bufs=3))
        hpool = ctx.enter_context(tc.tile_pool(name="hp", bufs=3))
        spool = ctx.enter_context(tc.tile_pool(name="sp", bufs=2))
        pbig = ctx.enter_context(tc.tile_pool(name="pbig", bufs=8, space="PSUM"))

        # Weight-queue order = first-use order: the row-tiled remainder
        # weights open superchunk 0, then w1, then biases (first ReLU is
        # ~2us later), then the rest.
        w1r = consts.tile([128, 128], FP8, tag="w1r", name="w1r")
        nc.scalar.dma_start(w1r[:], w1r_d[:])
        w1 = consts.tile([128, K0F, H], FP8, tag="w1", name="w1")
        nc.scalar.dma_start(w1[:], w1_d[:])
        ball = consts.tile([128, 3 * KH + C], FP32, tag="ball", name="ball")
        nc.scalar.dma_start(ball[:], bal_d[:])
        w2 = consts.tile([128, KH, H], FP8, tag="w2", name="w2")
        nc.scalar.dma_start(w2[:], w2_d[:])
        w3 = consts.tile([128, KH, H], FP8, tag="w3", name="w3")
        nc.scalar.dma_start(w3[:], w3_d[:])
        w4 = consts.tile([128, KH, C], BF16, tag="w4", name="w4")
        nc.scalar.dma_start(w4[:], w4_d[:])

        b4s = ball[:, 3 * KH : 3 * KH + C]

        # PE warm-up: dummy matmuls during the initial DMA wait so the HAM
        # clock gate is at 2.4 GHz when real work arrives.
        warm = consts.tile([128, NB], FP8, tag="warm", name="warm")
        nc.vector.memset(warm[:], 1.0)
        psw = pbig.tile([128, NB], FP32, tag="ps", name="ps_warm")
        for i in range(6):
            nc.tensor.matmul(
                psw[:], lhsT=warm[:, 0:128], rhs=warm[:],
                start=(i == 0), stop=(i == 5),
            )

        # Persistent softmax state: ln+subtract are deferred to two bulk
        # epilogue calls, so ScalarE never swaps activation tables (RELU/EXP
        # share a set, LN does not) inside the steady-state loop.
        logits_all = consts.tile([128, NRG, C], FP32, tag="logits_all", name="logits_all")
        esum_all = consts.tile([128, NRG], FP32, tag="esum_all", name="esum_all")
        lns_all = consts.tile([128, NRG], FP32, tag="lns_all", name="lns_all")
        obuf = consts.tile([128, NRG, C], FP32, tag="obuf", name="obuf")

        def relu_half(ps, out, bias_ap, on_scalar):
            if on_scalar:
                nc.scalar.activation(out, ps[:], reluf, bias=bias_ap)
            else:
                nc.vector.tensor_scalar(out, ps[:], bias_ap, 0.0, add_op, max_op)

        def l4_matmuls(h3, ps4):
            for hb in range(HB):
                for mm in range(NB // 128):
                    r = hb * (NB // 128) + mm
                    ms = slice(mm * 128, (mm + 1) * 128)
                    for k in range(KH):
                        nc.tensor.matmul(
                            ps4[:, r, :], lhsT=h3[k][:, hb, ms], rhs=w4[:, k, :],
                            start=(k == 0), stop=(k == KH - 1),
                        )

        def l4_softmax_state(sc, ps4):
            # logits + exp + sum(exp) for superchunk sc.
            rg0 = sc * MG
            lg = logits_all[:, rg0 : rg0 + MG, :]
            nc.vector.tensor_tensor(
                lg, ps4[:, 0:MG, :],
                b4s[:, None, :].to_broadcast((128, MG, C)), add_op,
            )
            etile = spool.tile([128, MG, C], FP32, tag="etile", name="etile")
            nc.scalar.activation(etile[:], lg, expf)
            nc.vector.tensor_reduce(
                esum_all[:, rg0 : rg0 + MG], etile[:],
                axis=mybir.AxisListType.X, op=add_op,
            )

        def softmax_epilogue(rg0, rg1):
            # out = logits - ln(sum(exp(logits))) for row-groups [rg0, rg1)
            n = rg1 - rg0
            nc.scalar.activation(lns_all[:, rg0:rg1], esum_all[:, rg0:rg1], lnf)
            nc.vector.tensor_tensor(
                obuf[:, rg0:rg1, :], logits_all[:, rg0:rg1, :],
                lns_all[:, rg0:rg1, None].to_broadcast((128, n, C)), sub_op,
            )
            nc.sync.dma_start(out_d[:, rg0:rg1, :], obuf[:, rg0:rg1, :])

        h3_prev = None
        ps4_prev = None

        for sc in range(NCHUNK):
            xr = xpool.tile([128, SNB], FP8, tag="xr", name="xr")
            nc.sync.dma_start(xr[:], xr_d[sc * 128 : (sc + 1) * 128, :])
            xt = xpool.tile([128, K0F, SNB], FP8, tag="xt", name="xt")
            nc.sync.dma_start(xt[:], xm_d[sc * 128 : (sc + 1) * 128, :])

            # Layer 1 [784 -> 512], one batch-half at a time: the K=16
            # remainder opens all 4 m-groups concurrently (distinct PE
            # row-groups), then 3 fp8-DoubleRow slots per m.
            h1p = [
                hpool.tile([128, 2, HB, NB], FP8, tag=f"h1p_{j}", name=f"h1p_{j}")
                for j in range(KH // 2)
            ]
            for hb in range(HB):
                bsl = slice(hb * NB, (hb + 1) * NB)
                ps1 = [
                    pbig.tile([128, NB], FP32, tag="ps", name=f"ps1_{m}_{hb}")
                    for m in range(KH)
                ]
                for m in range(KH):
                    nc.tensor.matmul(
                        ps1[m][:], lhsT=w1r[32 * m : 32 * m + K0R, :],
                        rhs=xr[32 * m : 32 * m + K0R, bsl],
                        start=True, stop=False, perf_mode=None,
                        tile_position=(32 * m, 0),
                    )
                for m in range(KH):
                    ms = slice(m * 128, (m + 1) * 128)
                    for k in range(0, K0F, 2):
                        nc.tensor.matmul(
                            ps1[m][:], lhsT=w1[:, k : k + 2, ms],
                            rhs=xt[:, k : k + 2, bsl],
                            start=False, stop=(k == K0F - 2), perf_mode=drow,
                        )
                    relu_half(
                        ps1[m], h1p[m // 2][:, m % 2, hb, :],
                        ball[:, m : m + 1], on_scalar=((m + hb) % 2 == 0),
                    )

            # Layer 4 of the previous superchunk (its inputs are long ready).
            if h3_prev is not None:
                l4_matmuls(h3_prev, ps4_prev)

            def hidden_layer(w, src, dsts, bias_base, out_of_h3):
                for hb in range(HB):
                    ps = [
                        pbig.tile([128, NB], FP32, tag="ps", name=f"psh_{m}_{hb}")
                        for m in range(KH)
                    ]
                    for m in range(KH):
                        ms = slice(m * 128, (m + 1) * 128)
                        for j in range(KH // 2):
                            nc.tensor.matmul(
                                ps[m][:], lhsT=w[:, 2 * j : 2 * j + 2, ms],
                                rhs=src[j][:, :, hb, :],
                                start=(j == 0), stop=(j == KH // 2 - 1),
                                perf_mode=drow,
                            )
                        out = (
                            dsts[m][:, hb, :] if out_of_h3
                            else dsts[m // 2][:, m % 2, hb, :]
                        )
                        relu_half(
                            ps[m], out, ball[:, bias_base + m : bias_base + m + 1],
                            on_scalar=((m + hb) % 2 == 0),
                        )
                    if hb == 0 and out_of_h3 is False and h3_prev is not None:
                        # exp/sum(exp) of the previous superchunk: issued
                        # mid-superchunk so its ScalarE/DVE ops never queue
                        # ahead of ReLUs the PE is about to wait on.
                        l4_softmax_state(sc - 1, ps4_prev)

            # Layer 2 [512 -> 512]
            h2p = [
                hpool.tile([128, 2, HB, NB], FP8, tag=f"h2p_{j}", name=f"h2p_{j}")
                for j in range(KH // 2)
            ]
            hidden_layer(w2, h1p, h2p, KH, out_of_h3=False)

            # Layer 3 [512 -> 512], bf16 out (layer-4 lhsT)
            h3 = [
                hpool.tile([128, HB, NB], BF16, tag=f"h3_{m}", name=f"h3_{m}")
                for m in range(KH)
            ]
            hidden_layer(w3, h2p, h3, 2 * KH, out_of_h3=True)

            if sc == NCHUNK - 2:
                # Bulk ln+subtract+store for superchunks 0-5 hides under the
                # last superchunk's matmuls.
                softmax_epilogue(0, (NCHUNK - 2) * MG)

            h3_prev = h3
            ps4_prev = pbig.tile([128, MG, C], FP32, tag="ps", name="ps4")

        l4_matmuls(h3_prev, ps4_prev)
        l4_softmax_state(NCHUNK - 1, ps4_prev)
        softmax_epilogue((NCHUNK - 2) * MG, NRG)

    nc.compile()
    return nc


def _get_nc():
    global _CACHED_NC
    if _CACHED_NC is None:
        _CACHED_NC = build_nc()
    return _CACHED_NC


def make_in_maps(x, W1, b1, W2, b2, W3, b3, W4, b4):
    bf16 = ml_dtypes.bfloat16
    fp8 = ml_dtypes.float8_e4m3
    f32 = np.float32
    W1, W2, W3, W4 = (np.asarray(w, dtype=f32) for w in (W1, W2, W3, W4))

    # w1p[p, k*512+m] = W1[m, k*128+p]
    w1p = np.ascontiguousarray(
        W1[:, : K0F * 128].reshape(H, K0F, 128).transpose(2, 1, 0)
    ).reshape(128, K0F * H).astype(fp8)
    # w1r[32i+j, c] = W1[128i+c, 768+j]  (row-tiled remainder weights)
    w1r = np.zeros((128, 128), dtype=fp8)
    wr = W1[:, K0F * 128 :].astype(fp8)  # [512, 16]
    for i in range(KH):
        w1r[32 * i : 32 * i + K0R, :] = wr[128 * i : 128 * (i + 1), :].T
    # w2p[p, o*512+m] = W2[m, o*128+p]
    def packw(W):
        return np.ascontiguousarray(
            W.T.reshape(KH, 128, H).transpose(1, 0, 2)
        ).reshape(128, KH * H).astype(fp8)
    w2p, w3p = packw(W2), packw(W3)
    w4p = np.ascontiguousarray(
        W4.T.reshape(KH, 128, C).transpose(1, 0, 2)
    ).reshape(128, KH * C).astype(bf16)
    ball = np.concatenate(
        [
            np.asarray(b1, f32).reshape(KH, 128).T,
            np.asarray(b2, f32).reshape(KH, 128).T,
            np.asarray(b3, f32).reshape(KH, 128).T,
            np.tile(np.asarray(b4, f32)[None, :], (128, 1)),
        ],
        axis=1,
    )
    common = {
        "w1p": w1p, "w1r": w1r, "w2p": w2p, "w3p": w3p, "w4p": w4p,
        "ball": np.ascontiguousarray(ball),
    }

    xq = np.asarray(x).astype(fp8)
    in_maps = []
    for ci in range(N_CORES):
        xs = xq[ci * BC : (ci + 1) * BC]  # [8192, 784]
        # xmain[sc*128+p, k*1024+b] = xs[sc*1024+b, k*128+p]
        xmain = np.ascontiguousarray(
            xs[:, : K0F * 128].reshape(NCHUNK, SNB, K0F, 128).transpose(0, 3, 2, 1)
        ).reshape(NCHUNK * 128, K0F * SNB)
        # xrem[sc*128+32i+j, b] = xs[sc*1024+b, 768+j], replicated over i
        xrp = xs[:, K0F * 128 :].reshape(NCHUNK, SNB, K0R).transpose(0, 2, 1)
        xrem = np.zeros((NCHUNK, 128, SNB), dtype=fp8)
        for i in range(KH):
            xrem[:, 32 * i : 32 * i + K0R, :] = xrp
        in_maps.append(
            {"xmain": xmain, "xrem": xrem.reshape(NCHUNK * 128, SNB), **common}
        )
    return in_maps


def assemble_output(res):
    # out dram is the flat SBUF layout [128, 64, 10]; row rg*128+p of the
    # core's shard lives at out[p, rg, :].
    parts = []
    for i in range(N_CORES):
        o = np.asarray(res.results[i]["out"], dtype=np.float32)
        parts.append(o.transpose(1, 0, 2).reshape(BC, C))
    return np.concatenate(parts, axis=0)


def kernel(x, W1, b1, W2, b2, W3, b3, W4, b4):
    in_maps = make_in_maps(x, W1, b1, W2, b2, W3, b3, W4, b4)
    nc = _get_nc()
    res = run_bass_kernel_spmd(nc, in_maps, list(range(N_CORES)))
    return assemble_output(res)
